# revision 27
# baseline (speedup 1.0000x reference)
"""Grouped SwiGLU FFN (8 experts) — expert-parallel Bass kernel for 8 trn2 cores.

Per core (one expert): out = (silu(x@w1) * (x@w3T)) @ w2T.
  x: [T=1024, D=2048], w1: [D, H=4096], w3: [H, D], w2: [D, H].

All matmul operands are bf16 (PE runs bf16 at the same 1 col/cycle rate as
fp32r, but weight DMA halves and FWL doubles LDWEIGHTS rate); PSUM
accumulation is fp32, epilogues in fp32. End-to-end rel err ~4e-3.

Device-side (layouts pre-packed on host, zero on-device transposes):
  phase1: g^T[h, t]  = silu(w1-tile.T @ x^T) * (w3-tile.T @ x^T)  per h-tile,
          full H in one pass (g is [128, 32, 1024] bf16 = 64KB/partition)
  phase2: out^T[d,t] = sum over all 32 h-tiles of w2-tile.T @ g^T, one psum
          accumulation per (dtt, t-half); result copied + DMAd per dtt.
Matmuls are t-half-paired so each weight tile is stationary for 2 MMs.
The first 4 dt-tiles of ht=0 are split into N=128 quarter-MMs to densify
early PE activity (HAM un-throttles ~3.4us sooner).
"""

import sys

sys.path.insert(0, "/opt/trn_rl_repo")

import numpy as np
import ml_dtypes

import concourse.bass as bass
from concourse import bacc
import concourse.mybir as mybir
import concourse.tile as tile
from concourse.bass_utils import run_bass_kernel_spmd

E, T, D, H = 8, 1024, 2048, 4096
P = 128
NT = 512            # matmul moving free dim per psum bank (fp32 psum)
DT = D // P         # 16 contraction tiles over D
HT = H // P         # 32 h-tiles
TH = T // NT        # 2 t-halves
DTT = D // P        # 16 out^T row tiles
F32 = mybir.dt.float32
BF16 = mybir.dt.bfloat16

_CACHE: dict = {}


def _build_nc():
    nc = bacc.Bacc("TRN2", target_bir_lowering=False, debug=False)
    xp = nc.dram_tensor("xp", [P, DT, T], BF16, kind="ExternalInput")
    # w1 and w3 packed together: one DMA issue (~620ns of sync-engine time
    # each) covers both weight tiles of an ht
    wp = nc.dram_tensor("wp", [HT, P, 2, DT, P], BF16, kind="ExternalInput")
    w2p = nc.dram_tensor("w2p", [DTT, P, HT, P], BF16, kind="ExternalInput")
    outT = nc.dram_tensor("outT", [D, T], F32, kind="ExternalOutput")

    with tile.TileContext(nc) as tc:
        with (
            tc.tile_pool(name="xpool", bufs=1) as xpool,
            tc.tile_pool(name="gpool", bufs=1) as gpool,
            tc.tile_pool(name="wpool", bufs=2) as wpool,
            tc.tile_pool(name="w2pool", bufs=2) as w2pool,
            tc.tile_pool(name="spool", bufs=2) as spool,
            tc.tile_pool(name="stpool", bufs=2) as stpool,
            tc.tile_pool(name="pspool", bufs=8, space="PSUM") as pspool,
        ):
            def load_w(ht, chunk=None):
                if chunk is None:
                    wsb = wpool.tile([P, 2, DT, P], BF16, tag="w", name=f"wsb_{ht}")
                    nc.sync.dma_start(wsb, wp[ht])
                    return wsb
                wsb, sl = chunk
                nc.sync.dma_start(wsb[:, :, sl], wp[ht, :, :, sl])
                return wsb

            # startup: DMA issue is ~620ns/instr on the sync queue and each
            # queue sustains ~100 GB/s, so the first chunks are small (fast
            # first arrival) and sizes grow as the PE pipeline fills; weights
            # interleave with x in consumption order
            # PE pre-warm: garbage matmuls with no DMA dependency keep the
            # PE busy from ~6.3us so the HAM clock-gate opens (1.2->2.4GHz)
            # before the first real matmul's data has even arrived
            wsc = wpool.tile([P, P], BF16, tag="wrm")
            nc.vector.memset(wsc, 0.0)
            pwm = pspool.tile([P, P], F32, tag="wrm_ps", bufs=1)
            for _ in range(32):
                nc.tensor.matmul(
                    pwm, lhsT=wsc, rhs=wsc, start=True, stop=True,
                    skip_group_check=True,
                )

            xsb = xpool.tile([P, DT, T], BF16, tag="x")
            w_pre = wpool.tile([P, 2, DT, P], BF16, tag="w", name="wsb_0")
            xq = 0

            def issue_x(n):
                nonlocal xq
                sl = slice(xq, xq + n)
                nc.sync.dma_start(xsb[:, sl], xp[:, sl])
                xq += n

            def issue_w0(lo, hi):
                load_w(0, chunk=(w_pre, slice(lo, hi)))

            # head of x (dt 0-3) interleaves with w0 on the sync queue; the
            # tail (dt 4-15, 3MB) goes through the scalar engine's DMA path
            # so two hardware queues stream in parallel during the ramp
            issue_w0(0, 2)
            nc.sync.dma_start(xsb[:, 0, 0:NT], xp[:, 0, 0:NT])
            nc.sync.dma_start(xsb[:, 0, NT:T], xp[:, 0, NT:T])
            xq = 1
            issue_w0(2, 6)
            issue_x(1)
            issue_x(1)
            issue_w0(6, 11)
            issue_x(1)
            issue_w0(11, DT)
            w_nxt = load_w(1)
            for lo in range(4, DT, 4):
                nc.scalar.dma_start(
                    xsb[:, lo : lo + 4], xp[:, lo : lo + 4]
                )
            xq = DT

            g = gpool.tile([P, HT, T], BF16, tag="g")

            def mm_quarters(ps, wsb, dt_i, th, start):
                # N=128 quarter MMs: denser early PE stream (same psum bank,
                # only the group's very first MM carries start=True)
                for q in range(4):
                    qs = slice(th * NT + q * P, th * NT + (q + 1) * P)
                    ps_q = slice(q * P, (q + 1) * P)
                    nc.tensor.matmul(
                        ps[:, ps_q],
                        lhsT=wsb[:, dt_i],
                        rhs=xsb[:, dt_i, qs],
                        start=(start and q == 0),
                        stop=False,
                        skip_group_check=True,
                    )

            for ht in range(HT):
                if ht == 0:
                    wsb = w_pre
                elif ht == 1:
                    wsb = w_nxt
                else:
                    wsb = w_cur
                if 1 <= ht < HT - 1:
                    w_cur = load_w(ht + 1)
                w1sb, w3sb = wsb[:, 0], wsb[:, 1]
                ps1 = [
                    pspool.tile([P, NT], F32, tag="ps", bufs=7, name=f"ps1_{th}")
                    for th in range(TH)
                ]
                ps3 = [
                    pspool.tile([P, NT], F32, tag="ps", bufs=7, name=f"ps3_{th}")
                    for th in range(TH)
                ]
                # ht=0: all 4 groups advance per dt chunk, so each x chunk is
                # consumed once as it arrives (~290 GB/s demand, matches DMA).
                # ht>0: all w1 groups then all w3 — frees ps1 banks (silu-only
                # readers) early so the next ht never waits on psum rotation.
                if ht == 0:
                    for dt_i in range(DT):
                        if dt_i < 4:
                            # th-major so the first 8 quarter-MM bursts only
                            # need the t<512 half of the x chunk
                            for th in range(TH):
                                for ps_pair, wsb in ((ps1, w1sb), (ps3, w3sb)):
                                    mm_quarters(
                                        ps_pair[th], wsb, dt_i, th,
                                        start=(dt_i == 0),
                                    )
                            continue
                        for ps_pair, wsb in ((ps1, w1sb), (ps3, w3sb)):
                            for th in range(TH):
                                ts = slice(th * NT, (th + 1) * NT)
                                nc.tensor.matmul(
                                    ps_pair[th],
                                    lhsT=wsb[:, dt_i],
                                    rhs=xsb[:, dt_i, ts],
                                    start=(dt_i == 0),
                                    stop=(dt_i == DT - 1),
                                    skip_group_check=True,
                                )
                else:
                    for ps_pair, wsb in ((ps1, w1sb), (ps3, w3sb)):
                        for dt_i in range(DT):
                            for th in range(TH):
                                ts = slice(th * NT, (th + 1) * NT)
                                nc.tensor.matmul(
                                    ps_pair[th],
                                    lhsT=wsb[:, dt_i],
                                    rhs=xsb[:, dt_i, ts],
                                    start=(dt_i == 0),
                                    stop=(dt_i == DT - 1),
                                )
                for th in range(TH):
                    ts = slice(th * NT, (th + 1) * NT)
                    sil = spool.tile([P, NT], F32, tag="sil")
                    nc.scalar.activation(
                        sil, ps1[th], mybir.ActivationFunctionType.Silu
                    )
                    nc.vector.tensor_mul(out=g[:, ht, ts], in0=sil, in1=ps3[th])

            # phase 2: out^T[dtt] = sum_ht w2tile.T @ g, full-K psum groups
            for dtt in range(DTT):
                w2sb = w2pool.tile([P, HT, P], BF16, tag="w2")
                nc.sync.dma_start(w2sb, w2p[dtt])
                po = [
                    pspool.tile([P, NT], F32, tag="ps", bufs=7, name=f"po_{th}")
                    for th in range(TH)
                ]
                for ht in range(HT):
                    for th in range(TH):
                        ts = slice(th * NT, (th + 1) * NT)
                        nc.tensor.matmul(
                            po[th],
                            lhsT=w2sb[:, ht],
                            rhs=g[:, ht, ts],
                            start=(ht == 0),
                            stop=(ht == HT - 1),
                        )
                stage = stpool.tile([P, T], F32, tag="stage")
                # drain the two t-halves on both engines in parallel
                nc.scalar.copy(stage[:, 0:NT], po[0])
                nc.vector.tensor_copy(out=stage[:, NT:T], in_=po[1])
                dsl = slice(dtt * P, (dtt + 1) * P)
                if dtt < DTT - 1:
                    nc.sync.dma_start(outT[dsl], stage)
                else:
                    # split the last DMA so the tail drains in halves
                    nc.sync.dma_start(outT[dsl, 0:NT], stage[:, 0:NT])
                    nc.sync.dma_start(outT[dsl, NT:T], stage[:, NT:T])
    nc.compile()
    return nc


def _pack_inputs(x, w1, w2, w3):
    """Per-expert host-side packing into DMA-linear bf16 layouts."""
    bf = ml_dtypes.bfloat16
    in_maps = []
    for e in range(E):
        xe = np.asarray(x[e], dtype=np.float32).astype(bf)
        w1e = np.asarray(w1[e], dtype=np.float32).astype(bf)
        w2e = np.asarray(w2[e], dtype=np.float32).astype(bf)
        w3e = np.asarray(w3[e], dtype=np.float32).astype(bf)
        # xp[p, dt, t] = x[t, dt*128+p]  (matches the xsb SBUF layout)
        xpk = np.ascontiguousarray(xe.reshape(T, DT, P).transpose(2, 1, 0))
        # wp[ht, p, 0, dt, h] = w1[dt*128+p, ht*128+h]
        # wp[ht, p, 1, dt, h] = w3[ht*128+h, dt*128+p]
        w1pk = w1e.reshape(DT, P, HT, P).transpose(2, 1, 0, 3)
        w3pk = w3e.reshape(HT, P, DT, P).transpose(0, 3, 2, 1)
        wpk = np.ascontiguousarray(np.stack([w1pk, w3pk], axis=2))
        # w2p[dtt, p, ht, d] = w2[dtt*128+d, ht*128+p]  (partition-first tile)
        w2pk = np.ascontiguousarray(
            w2e.reshape(DTT, P, HT, P).transpose(0, 3, 2, 1)
        )
        in_maps.append({"xp": xpk, "wp": wpk, "w2p": w2pk})
    return in_maps


def kernel(x, w1, w2, w3, _trace=False, _trace_kwargs=None):
    if "nc" not in _CACHE:
        _CACHE["nc"] = _build_nc()
    nc = _CACHE["nc"]
    in_maps = _pack_inputs(x, w1, w2, w3)
    kw = {}
    if _trace:
        kw = {"trace": True}
        if _trace_kwargs:
            kw.update(_trace_kwargs)
    res = run_bass_kernel_spmd(nc, in_maps, core_ids=list(range(E)), **kw)
    out = np.empty((E, T, D), dtype=np.float32)
    for e in range(E):
        out[e] = res.results[e]["outT"].T
    if _trace:
        _CACHE["last_results"] = res
    return out


# revision 31
# speedup vs baseline: 1.0041x; 1.0041x over previous
"""Grouped SwiGLU FFN (8 experts) — expert-parallel Bass kernel for 8 trn2 cores.

Per core (one expert): out = (silu(x@w1) * (x@w3T)) @ w2T.
  x: [T=1024, D=2048], w1: [D, H=4096], w3: [H, D], w2: [D, H].

All matmul operands are bf16 (PE runs bf16 at the same 1 col/cycle rate as
fp32r, but weight DMA halves and FWL doubles LDWEIGHTS rate); PSUM
accumulation is fp32, epilogues in fp32. End-to-end rel err ~4e-3.

Device-side (layouts pre-packed on host, zero on-device transposes):
  phase1: g^T[h, t]  = silu(w1-tile.T @ x^T) * (w3-tile.T @ x^T)  per h-tile,
          full H in one pass (g is [128, 32, 1024] bf16 = 64KB/partition)
  phase2: out^T[d,t] = sum over all 32 h-tiles of w2-tile.T @ g^T, one psum
          accumulation per (dtt, t-half); result copied + DMAd per dtt.
Matmuls are t-half-paired so each weight tile is stationary for 2 MMs.
The first 4 dt-tiles of ht=0 are split into N=128 quarter-MMs to densify
early PE activity (HAM un-throttles ~3.4us sooner).
"""

import sys

sys.path.insert(0, "/opt/trn_rl_repo")

import numpy as np
import ml_dtypes

import concourse.bass as bass
from concourse import bacc
import concourse.mybir as mybir
import concourse.tile as tile
from concourse.bass_utils import run_bass_kernel_spmd

E, T, D, H = 8, 1024, 2048, 4096
P = 128
NT = 512            # matmul moving free dim per psum bank (fp32 psum)
DT = D // P         # 16 contraction tiles over D
HT = H // P         # 32 h-tiles
TH = T // NT        # 2 t-halves
DTT = D // P        # 16 out^T row tiles
F32 = mybir.dt.float32
BF16 = mybir.dt.bfloat16

_CACHE: dict = {}


def _build_nc():
    nc = bacc.Bacc("TRN2", target_bir_lowering=False, debug=False)
    xp = nc.dram_tensor("xp", [P, DT, T], BF16, kind="ExternalInput")
    # w1 and w3 packed together: one DMA issue (~620ns of sync-engine time
    # each) covers both weight tiles of an ht
    wp = nc.dram_tensor("wp", [HT, P, 2, DT, P], BF16, kind="ExternalInput")
    w2p = nc.dram_tensor("w2p", [DTT, P, HT, P], BF16, kind="ExternalInput")
    outT = nc.dram_tensor("outT", [D, T], BF16, kind="ExternalOutput")

    with tile.TileContext(nc) as tc:
        with (
            tc.tile_pool(name="xpool", bufs=1) as xpool,
            tc.tile_pool(name="gpool", bufs=1) as gpool,
            tc.tile_pool(name="wpool", bufs=2) as wpool,
            tc.tile_pool(name="w2pool", bufs=2) as w2pool,
            tc.tile_pool(name="spool", bufs=2) as spool,
            tc.tile_pool(name="stpool", bufs=2) as stpool,
            tc.tile_pool(name="pspool", bufs=8, space="PSUM") as pspool,
        ):
            def load_w(ht, chunk=None):
                if chunk is None:
                    wsb = wpool.tile([P, 2, DT, P], BF16, tag="w", name=f"wsb_{ht}")
                    nc.sync.dma_start(wsb, wp[ht])
                    return wsb
                wsb, sl = chunk
                nc.sync.dma_start(wsb[:, :, sl], wp[ht, :, :, sl])
                return wsb

            # startup: DMA issue is ~620ns/instr on the sync queue and each
            # queue sustains ~100 GB/s, so the first chunks are small (fast
            # first arrival) and sizes grow as the PE pipeline fills; weights
            # interleave with x in consumption order
            # PE pre-warm: garbage matmuls with no DMA dependency keep the
            # PE busy from ~6.3us so the HAM clock-gate opens (1.2->2.4GHz)
            # before the first real matmul's data has even arrived
            wsc = wpool.tile([P, P], BF16, tag="wrm")
            nc.vector.memset(wsc, 0.0)
            pwm = pspool.tile([P, P], F32, tag="wrm_ps", bufs=1)
            for _ in range(32):
                nc.tensor.matmul(
                    pwm, lhsT=wsc, rhs=wsc, start=True, stop=True,
                    skip_group_check=True,
                )

            xsb = xpool.tile([P, DT, T], BF16, tag="x")
            w_pre = wpool.tile([P, 2, DT, P], BF16, tag="w", name="wsb_0")
            xq = 0

            def issue_x(n):
                nonlocal xq
                sl = slice(xq, xq + n)
                nc.sync.dma_start(xsb[:, sl], xp[:, sl])
                xq += n

            def issue_w0(lo, hi):
                load_w(0, chunk=(w_pre, slice(lo, hi)))

            issue_w0(0, 2)
            nc.sync.dma_start(xsb[:, 0, 0:NT], xp[:, 0, 0:NT])
            nc.sync.dma_start(xsb[:, 0, NT:T], xp[:, 0, NT:T])
            xq = 1
            issue_w0(2, 6)
            issue_x(1)
            issue_x(1)
            issue_w0(6, 11)
            issue_x(1)
            issue_x(1)
            issue_w0(11, DT)
            issue_x(1)
            issue_x(1)
            issue_x(1)
            w_nxt = load_w(1)
            while xq < DT:
                issue_x(2)

            g = gpool.tile([P, HT, T], BF16, tag="g")

            def mm_quarters(ps, wsb, dt_i, th, start):
                # N=128 quarter MMs: denser early PE stream (same psum bank,
                # only the group's very first MM carries start=True)
                for q in range(4):
                    qs = slice(th * NT + q * P, th * NT + (q + 1) * P)
                    ps_q = slice(q * P, (q + 1) * P)
                    nc.tensor.matmul(
                        ps[:, ps_q],
                        lhsT=wsb[:, dt_i],
                        rhs=xsb[:, dt_i, qs],
                        start=(start and q == 0),
                        stop=False,
                        skip_group_check=True,
                    )

            for ht in range(HT):
                if ht == 0:
                    wsb = w_pre
                elif ht == 1:
                    wsb = w_nxt
                else:
                    wsb = w_cur
                if 1 <= ht < HT - 1:
                    w_cur = load_w(ht + 1)
                w1sb, w3sb = wsb[:, 0], wsb[:, 1]
                ps1 = [
                    pspool.tile([P, NT], F32, tag="ps", bufs=7, name=f"ps1_{th}")
                    for th in range(TH)
                ]
                ps3 = [
                    pspool.tile([P, NT], F32, tag="ps", bufs=7, name=f"ps3_{th}")
                    for th in range(TH)
                ]
                # ht=0: all 4 groups advance per dt chunk, so each x chunk is
                # consumed once as it arrives (~290 GB/s demand, matches DMA).
                # ht>0: all w1 groups then all w3 — frees ps1 banks (silu-only
                # readers) early so the next ht never waits on psum rotation.
                if ht == 0:
                    for dt_i in range(DT):
                        if dt_i < 4:
                            # th-major so the first 8 quarter-MM bursts only
                            # need the t<512 half of the x chunk
                            for th in range(TH):
                                for ps_pair, wsb in ((ps1, w1sb), (ps3, w3sb)):
                                    mm_quarters(
                                        ps_pair[th], wsb, dt_i, th,
                                        start=(dt_i == 0),
                                    )
                            continue
                        for ps_pair, wsb in ((ps1, w1sb), (ps3, w3sb)):
                            for th in range(TH):
                                ts = slice(th * NT, (th + 1) * NT)
                                nc.tensor.matmul(
                                    ps_pair[th],
                                    lhsT=wsb[:, dt_i],
                                    rhs=xsb[:, dt_i, ts],
                                    start=(dt_i == 0),
                                    stop=(dt_i == DT - 1),
                                    skip_group_check=True,
                                )
                else:
                    for ps_pair, wsb in ((ps1, w1sb), (ps3, w3sb)):
                        for dt_i in range(DT):
                            for th in range(TH):
                                ts = slice(th * NT, (th + 1) * NT)
                                nc.tensor.matmul(
                                    ps_pair[th],
                                    lhsT=wsb[:, dt_i],
                                    rhs=xsb[:, dt_i, ts],
                                    start=(dt_i == 0),
                                    stop=(dt_i == DT - 1),
                                )
                for th in range(TH):
                    ts = slice(th * NT, (th + 1) * NT)
                    sil = spool.tile([P, NT], F32, tag="sil")
                    nc.scalar.activation(
                        sil, ps1[th], mybir.ActivationFunctionType.Silu
                    )
                    nc.vector.tensor_mul(out=g[:, ht, ts], in0=sil, in1=ps3[th])

            # phase 2: out^T[dtt] = sum_ht w2tile.T @ g, full-K psum groups
            for dtt in range(DTT):
                w2sb = w2pool.tile([P, HT, P], BF16, tag="w2")
                nc.sync.dma_start(w2sb, w2p[dtt])
                po = [
                    pspool.tile([P, NT], F32, tag="ps", bufs=7, name=f"po_{th}")
                    for th in range(TH)
                ]
                for ht in range(HT):
                    for th in range(TH):
                        ts = slice(th * NT, (th + 1) * NT)
                        nc.tensor.matmul(
                            po[th],
                            lhsT=w2sb[:, ht],
                            rhs=g[:, ht, ts],
                            start=(ht == 0),
                            stop=(ht == HT - 1),
                        )
                stage = stpool.tile([P, T], BF16, tag="stage")
                # drain the two t-halves on both engines in parallel
                nc.scalar.copy(stage[:, 0:NT], po[0])
                nc.vector.tensor_copy(out=stage[:, NT:T], in_=po[1])
                dsl = slice(dtt * P, (dtt + 1) * P)
                if dtt < DTT - 1:
                    nc.sync.dma_start(outT[dsl], stage)
                else:
                    # split the last DMA so the tail drains in halves
                    nc.sync.dma_start(outT[dsl, 0:NT], stage[:, 0:NT])
                    nc.sync.dma_start(outT[dsl, NT:T], stage[:, NT:T])
    nc.compile()
    return nc


def _pack_inputs(x, w1, w2, w3):
    """Per-expert host-side packing into DMA-linear bf16 layouts."""
    bf = ml_dtypes.bfloat16
    in_maps = []
    for e in range(E):
        xe = np.asarray(x[e], dtype=np.float32).astype(bf)
        w1e = np.asarray(w1[e], dtype=np.float32).astype(bf)
        w2e = np.asarray(w2[e], dtype=np.float32).astype(bf)
        w3e = np.asarray(w3[e], dtype=np.float32).astype(bf)
        # xp[p, dt, t] = x[t, dt*128+p]  (matches the xsb SBUF layout)
        xpk = np.ascontiguousarray(xe.reshape(T, DT, P).transpose(2, 1, 0))
        # wp[ht, p, 0, dt, h] = w1[dt*128+p, ht*128+h]
        # wp[ht, p, 1, dt, h] = w3[ht*128+h, dt*128+p]
        w1pk = w1e.reshape(DT, P, HT, P).transpose(2, 1, 0, 3)
        w3pk = w3e.reshape(HT, P, DT, P).transpose(0, 3, 2, 1)
        wpk = np.ascontiguousarray(np.stack([w1pk, w3pk], axis=2))
        # w2p[dtt, p, ht, d] = w2[dtt*128+d, ht*128+p]  (partition-first tile)
        w2pk = np.ascontiguousarray(
            w2e.reshape(DTT, P, HT, P).transpose(0, 3, 2, 1)
        )
        in_maps.append({"xp": xpk, "wp": wpk, "w2p": w2pk})
    return in_maps


def kernel(x, w1, w2, w3, _trace=False, _trace_kwargs=None):
    if "nc" not in _CACHE:
        _CACHE["nc"] = _build_nc()
    nc = _CACHE["nc"]
    in_maps = _pack_inputs(x, w1, w2, w3)
    kw = {}
    if _trace:
        kw = {"trace": True}
        if _trace_kwargs:
            kw.update(_trace_kwargs)
    res = run_bass_kernel_spmd(nc, in_maps, core_ids=list(range(E)), **kw)
    out = np.empty((E, T, D), dtype=np.float32)
    for e in range(E):
        out[e] = res.results[e]["outT"].T.astype(np.float32)
    if _trace:
        _CACHE["last_results"] = res
    return out


# revision 32
# speedup vs baseline: 1.0047x; 1.0006x over previous
"""Grouped SwiGLU FFN (8 experts) — expert-parallel Bass kernel for 8 trn2 cores.

Per core (one expert): out = (silu(x@w1) * (x@w3T)) @ w2T.
  x: [T=1024, D=2048], w1: [D, H=4096], w3: [H, D], w2: [D, H].

All matmul operands are bf16 (PE runs bf16 at the same 1 col/cycle rate as
fp32r, but weight DMA halves and FWL doubles LDWEIGHTS rate); PSUM
accumulation is fp32, epilogues in fp32, output bf16. Rel err ~5e-3 vs the
fp32 reference (gate is 2e-2).

Device-side (layouts pre-packed on host, zero on-device transposes):
  phase1: g^T[h, t]  = silu(w1-tile.T @ x^T) * (w3-tile.T @ x^T)  per h-tile,
          full H in one pass (g is [128, 32, 1024] bf16 = 64KB/partition)
  phase2: out^T[d,t] = sum over all 32 h-tiles of w2-tile.T @ g^T, one psum
          accumulation per (dtt, t-half); result copied + DMAd per dtt.
Matmuls are t-half-paired so each weight tile is stationary for 2 MMs.

Startup choreography (the DMA issue stream is serialized ~620ns/instr onto
one hardware queue whose bandwidth ramps while all 8 cores contend for HBM):
garbage pre-warm matmuls trip the HAM clock gate (1.2->2.4GHz) before real
data lands; ht=0 weights+x stream in exact consumption order in graduated
chunk sizes; ht=0 advances all 4 psum groups per dt chunk; the first 4
dt-tiles run as N=128 quarter-MMs for a denser early PE stream.
"""

import sys

sys.path.insert(0, "/opt/trn_rl_repo")

import numpy as np
import ml_dtypes

import concourse.bass as bass
from concourse import bacc
import concourse.mybir as mybir
import concourse.tile as tile
from concourse.bass_utils import run_bass_kernel_spmd

E, T, D, H = 8, 1024, 2048, 4096
P = 128
NT = 512            # matmul moving free dim per psum bank (fp32 psum)
DT = D // P         # 16 contraction tiles over D
HT = H // P         # 32 h-tiles
TH = T // NT        # 2 t-halves
DTT = D // P        # 16 out^T row tiles
F32 = mybir.dt.float32
BF16 = mybir.dt.bfloat16

_CACHE: dict = {}


def _build_nc():
    nc = bacc.Bacc("TRN2", target_bir_lowering=False, debug=False)
    xp = nc.dram_tensor("xp", [P, DT, T], BF16, kind="ExternalInput")
    # w1 and w3 packed together: one DMA issue (~620ns of sync-engine time
    # each) covers both weight tiles of an ht
    wp = nc.dram_tensor("wp", [HT, P, 2, DT, P], BF16, kind="ExternalInput")
    w2p = nc.dram_tensor("w2p", [DTT, P, HT, P], BF16, kind="ExternalInput")
    outT = nc.dram_tensor("outT", [D, T], BF16, kind="ExternalOutput")

    with tile.TileContext(nc) as tc:
        with (
            tc.tile_pool(name="xpool", bufs=1) as xpool,
            tc.tile_pool(name="gpool", bufs=1) as gpool,
            tc.tile_pool(name="wpool", bufs=2) as wpool,
            tc.tile_pool(name="w2pool", bufs=2) as w2pool,
            tc.tile_pool(name="spool", bufs=2) as spool,
            tc.tile_pool(name="stpool", bufs=2) as stpool,
            tc.tile_pool(name="pspool", bufs=8, space="PSUM") as pspool,
        ):
            def load_w(ht, chunk=None):
                if chunk is None:
                    wsb = wpool.tile([P, 2, DT, P], BF16, tag="w", name=f"wsb_{ht}")
                    nc.sync.dma_start(wsb, wp[ht])
                    return wsb
                wsb, sl = chunk
                nc.sync.dma_start(wsb[:, :, sl], wp[ht, :, :, sl])
                return wsb

            # startup: DMA issue is ~620ns/instr on the sync queue and each
            # queue sustains ~100 GB/s, so the first chunks are small (fast
            # first arrival) and sizes grow as the PE pipeline fills; weights
            # interleave with x in consumption order
            # PE pre-warm: garbage matmuls with no DMA dependency keep the
            # PE busy from ~6.3us so the HAM clock-gate opens (1.2->2.4GHz)
            # before the first real matmul's data has even arrived
            wsc = wpool.tile([P, P], BF16, tag="wrm")
            nc.vector.memset(wsc, 0.0)
            pwm = pspool.tile([P, P], F32, tag="wrm_ps", bufs=1)
            for _ in range(32):
                nc.tensor.matmul(
                    pwm, lhsT=wsc, rhs=wsc, start=True, stop=True,
                    skip_group_check=True,
                )

            xsb = xpool.tile([P, DT, T], BF16, tag="x")
            w_pre = wpool.tile([P, 2, DT, P], BF16, tag="w", name="wsb_0")
            xq = 0

            def issue_x(n):
                nonlocal xq
                sl = slice(xq, xq + n)
                nc.sync.dma_start(xsb[:, sl], xp[:, sl])
                xq += n

            def issue_w0(lo, hi):
                load_w(0, chunk=(w_pre, slice(lo, hi)))

            issue_w0(0, 2)
            nc.sync.dma_start(xsb[:, 0, 0:NT], xp[:, 0, 0:NT])
            nc.sync.dma_start(xsb[:, 0, NT:T], xp[:, 0, NT:T])
            xq = 1
            issue_w0(2, 6)
            issue_x(1)
            issue_x(1)
            issue_w0(6, 11)
            issue_x(1)
            issue_x(1)
            issue_w0(11, DT)
            issue_x(1)
            issue_x(1)
            issue_x(1)
            w_nxt = load_w(1)
            while xq < DT:
                issue_x(2)

            g = gpool.tile([P, HT, T], BF16, tag="g")

            def mm_quarters(ps, wsb, dt_i, th, start):
                # N=128 quarter MMs: denser early PE stream (same psum bank,
                # only the group's very first MM carries start=True)
                for q in range(4):
                    qs = slice(th * NT + q * P, th * NT + (q + 1) * P)
                    ps_q = slice(q * P, (q + 1) * P)
                    nc.tensor.matmul(
                        ps[:, ps_q],
                        lhsT=wsb[:, dt_i],
                        rhs=xsb[:, dt_i, qs],
                        start=(start and q == 0),
                        stop=False,
                        skip_group_check=True,
                    )

            for ht in range(HT):
                if ht == 0:
                    wsb = w_pre
                elif ht == 1:
                    wsb = w_nxt
                else:
                    wsb = w_cur
                if 1 <= ht < HT - 1:
                    w_cur = load_w(ht + 1)
                w1sb, w3sb = wsb[:, 0], wsb[:, 1]
                ps1 = [
                    pspool.tile([P, NT], F32, tag="ps", bufs=7, name=f"ps1_{th}")
                    for th in range(TH)
                ]
                ps3 = [
                    pspool.tile([P, NT], F32, tag="ps", bufs=7, name=f"ps3_{th}")
                    for th in range(TH)
                ]
                # ht=0: all 4 groups advance per dt chunk, so each x chunk is
                # consumed once as it arrives (~290 GB/s demand, matches DMA).
                # ht>0: all w1 groups then all w3 — frees ps1 banks (silu-only
                # readers) early so the next ht never waits on psum rotation.
                if ht == 0:
                    for dt_i in range(DT):
                        if dt_i < 4:
                            # th-major so the first 8 quarter-MM bursts only
                            # need the t<512 half of the x chunk
                            for th in range(TH):
                                for ps_pair, wsb in ((ps1, w1sb), (ps3, w3sb)):
                                    mm_quarters(
                                        ps_pair[th], wsb, dt_i, th,
                                        start=(dt_i == 0),
                                    )
                            continue
                        for ps_pair, wsb in ((ps1, w1sb), (ps3, w3sb)):
                            for th in range(TH):
                                ts = slice(th * NT, (th + 1) * NT)
                                nc.tensor.matmul(
                                    ps_pair[th],
                                    lhsT=wsb[:, dt_i],
                                    rhs=xsb[:, dt_i, ts],
                                    start=(dt_i == 0),
                                    stop=(dt_i == DT - 1),
                                    skip_group_check=True,
                                )
                else:
                    for ps_pair, wsb in ((ps1, w1sb), (ps3, w3sb)):
                        for dt_i in range(DT):
                            for th in range(TH):
                                ts = slice(th * NT, (th + 1) * NT)
                                nc.tensor.matmul(
                                    ps_pair[th],
                                    lhsT=wsb[:, dt_i],
                                    rhs=xsb[:, dt_i, ts],
                                    start=(dt_i == 0),
                                    stop=(dt_i == DT - 1),
                                )
                for th in range(TH):
                    ts = slice(th * NT, (th + 1) * NT)
                    sil = spool.tile([P, NT], F32, tag="sil")
                    nc.scalar.activation(
                        sil, ps1[th], mybir.ActivationFunctionType.Silu
                    )
                    nc.vector.tensor_mul(out=g[:, ht, ts], in0=sil, in1=ps3[th])

            # phase 2: out^T[dtt] = sum_ht w2tile.T @ g, full-K psum groups
            for dtt in range(DTT):
                w2sb = w2pool.tile([P, HT, P], BF16, tag="w2")
                nc.sync.dma_start(w2sb, w2p[dtt])
                po = [
                    pspool.tile([P, NT], F32, tag="ps", bufs=7, name=f"po_{th}")
                    for th in range(TH)
                ]
                for ht in range(HT):
                    for th in range(TH):
                        ts = slice(th * NT, (th + 1) * NT)
                        nc.tensor.matmul(
                            po[th],
                            lhsT=w2sb[:, ht],
                            rhs=g[:, ht, ts],
                            start=(ht == 0),
                            stop=(ht == HT - 1),
                        )
                stage = stpool.tile([P, T], BF16, tag="stage")
                # drain the two t-halves on both engines in parallel
                nc.scalar.copy(stage[:, 0:NT], po[0])
                nc.vector.tensor_copy(out=stage[:, NT:T], in_=po[1])
                dsl = slice(dtt * P, (dtt + 1) * P)
                if dtt < DTT - 1:
                    nc.sync.dma_start(outT[dsl], stage)
                else:
                    # split the last DMA so the tail drains in halves
                    nc.sync.dma_start(outT[dsl, 0:NT], stage[:, 0:NT])
                    nc.sync.dma_start(outT[dsl, NT:T], stage[:, NT:T])
    nc.compile()
    return nc


def _pack_inputs(x, w1, w2, w3):
    """Per-expert host-side packing into DMA-linear bf16 layouts."""
    bf = ml_dtypes.bfloat16
    in_maps = []
    for e in range(E):
        xe = np.asarray(x[e], dtype=np.float32).astype(bf)
        w1e = np.asarray(w1[e], dtype=np.float32).astype(bf)
        w2e = np.asarray(w2[e], dtype=np.float32).astype(bf)
        w3e = np.asarray(w3[e], dtype=np.float32).astype(bf)
        # xp[p, dt, t] = x[t, dt*128+p]  (matches the xsb SBUF layout)
        xpk = np.ascontiguousarray(xe.reshape(T, DT, P).transpose(2, 1, 0))
        # wp[ht, p, 0, dt, h] = w1[dt*128+p, ht*128+h]
        # wp[ht, p, 1, dt, h] = w3[ht*128+h, dt*128+p]
        w1pk = w1e.reshape(DT, P, HT, P).transpose(2, 1, 0, 3)
        w3pk = w3e.reshape(HT, P, DT, P).transpose(0, 3, 2, 1)
        wpk = np.ascontiguousarray(np.stack([w1pk, w3pk], axis=2))
        # w2p[dtt, p, ht, d] = w2[dtt*128+d, ht*128+p]  (partition-first tile)
        w2pk = np.ascontiguousarray(
            w2e.reshape(DTT, P, HT, P).transpose(0, 3, 2, 1)
        )
        in_maps.append({"xp": xpk, "wp": wpk, "w2p": w2pk})
    return in_maps


def kernel(x, w1, w2, w3, _trace=False, _trace_kwargs=None):
    if "nc" not in _CACHE:
        _CACHE["nc"] = _build_nc()
    nc = _CACHE["nc"]
    in_maps = _pack_inputs(x, w1, w2, w3)
    kw = {}
    if _trace:
        kw = {"trace": True}
        if _trace_kwargs:
            kw.update(_trace_kwargs)
    res = run_bass_kernel_spmd(nc, in_maps, core_ids=list(range(E)), **kw)
    out = np.empty((E, T, D), dtype=np.float32)
    for e in range(E):
        out[e] = res.results[e]["outT"].T.astype(np.float32)
    if _trace:
        _CACHE["last_results"] = res
    return out


# revision 36
# speedup vs baseline: 1.0052x; 1.0006x over previous
"""Grouped SwiGLU FFN (8 experts) — expert-parallel Bass kernel for 8 trn2 cores.

Per core (one expert): out = (silu(x@w1) * (x@w3T)) @ w2T.
  x: [T=1024, D=2048], w1: [D, H=4096], w3: [H, D], w2: [D, H].

All matmul operands are bf16 (PE runs bf16 at the same 1 col/cycle rate as
fp32r, but weight DMA halves and FWL doubles LDWEIGHTS rate); PSUM
accumulation is fp32, epilogues and output in fp32. Rel err ~4e-3 vs the
fp32 reference (gate is 2e-2).

Device-side (layouts pre-packed on host, zero on-device transposes):
  phase1: g^T[h, t]  = silu(w1-tile.T @ x^T) * (w3-tile.T @ x^T)  per h-tile,
          full H in one pass (g is [128, 32, 1024] bf16 = 64KB/partition)
  phase2: out^T[d,t] = sum over all 32 h-tiles of w2-tile.T @ g^T, one psum
          accumulation per (dtt, t-half); result copied + DMAd per dtt.
Matmuls are t-half-paired so each weight tile is stationary for 2 MMs.

Startup choreography (the DMA issue stream is serialized ~620ns/instr onto
one hardware queue whose bandwidth ramps while all 8 cores contend for HBM):
garbage pre-warm matmuls trip the HAM clock gate (1.2->2.4GHz) before real
data lands; ht=0 weights+x stream in exact consumption order in graduated
chunk sizes; ht=0 advances all 4 psum groups per dt chunk; the first 4
dt-tiles run as N=128 quarter-MMs for a denser early PE stream.
"""

import sys

sys.path.insert(0, "/opt/trn_rl_repo")

import numpy as np
import ml_dtypes

import concourse.bass as bass
from concourse import bacc
import concourse.mybir as mybir
import concourse.tile as tile
from concourse.bass_utils import run_bass_kernel_spmd

E, T, D, H = 8, 1024, 2048, 4096
P = 128
NT = 512            # matmul moving free dim per psum bank (fp32 psum)
DT = D // P         # 16 contraction tiles over D
HT = H // P         # 32 h-tiles
TH = T // NT        # 2 t-halves
DTT = D // P        # 16 out^T row tiles
F32 = mybir.dt.float32
BF16 = mybir.dt.bfloat16

_CACHE: dict = {}


def _build_nc():
    nc = bacc.Bacc("TRN2", target_bir_lowering=False, debug=False)
    xp = nc.dram_tensor("xp", [P, DT, T], BF16, kind="ExternalInput")
    # w1 and w3 packed together: one DMA issue (~620ns of sync-engine time
    # each) covers both weight tiles of an ht
    wp = nc.dram_tensor("wp", [HT, P, 2, DT, P], BF16, kind="ExternalInput")
    w2p = nc.dram_tensor("w2p", [DTT, P, HT, P], BF16, kind="ExternalInput")
    outT = nc.dram_tensor("outT", [D, T], F32, kind="ExternalOutput")

    with tile.TileContext(nc) as tc:
        with (
            tc.tile_pool(name="xpool", bufs=1) as xpool,
            tc.tile_pool(name="gpool", bufs=1) as gpool,
            tc.tile_pool(name="wpool", bufs=2) as wpool,
            tc.tile_pool(name="w2pool", bufs=2) as w2pool,
            tc.tile_pool(name="spool", bufs=2) as spool,
            tc.tile_pool(name="stpool", bufs=2) as stpool,
            tc.tile_pool(name="pspool", bufs=8, space="PSUM") as pspool,
        ):
            def load_w(ht, chunk=None):
                if chunk is None:
                    wsb = wpool.tile([P, 2, DT, P], BF16, tag="w", name=f"wsb_{ht}")
                    nc.sync.dma_start(wsb, wp[ht])
                    return wsb
                wsb, sl = chunk
                nc.sync.dma_start(wsb[:, :, sl], wp[ht, :, :, sl])
                return wsb

            # startup: DMA issue is ~620ns/instr on the sync queue and each
            # queue sustains ~100 GB/s, so the first chunks are small (fast
            # first arrival) and sizes grow as the PE pipeline fills; weights
            # interleave with x in consumption order
            # PE pre-warm: garbage matmuls with no DMA dependency keep the
            # PE busy from ~6.3us so the HAM clock-gate opens (1.2->2.4GHz)
            # before the first real matmul's data has even arrived
            wsc = wpool.tile([P, P], BF16, tag="wrm")
            nc.vector.memset(wsc, 0.0)
            pwm = pspool.tile([P, P], F32, tag="wrm_ps", bufs=1)
            for _ in range(32):
                nc.tensor.matmul(
                    pwm, lhsT=wsc, rhs=wsc, start=True, stop=True,
                    skip_group_check=True,
                )

            xsb = xpool.tile([P, DT, T], BF16, tag="x")
            w_pre = wpool.tile([P, 2, DT, P], BF16, tag="w", name="wsb_0")
            xq = 0

            def issue_x(n):
                nonlocal xq
                sl = slice(xq, xq + n)
                nc.sync.dma_start(xsb[:, sl], xp[:, sl])
                xq += n

            def issue_w0(lo, hi):
                load_w(0, chunk=(w_pre, slice(lo, hi)))

            issue_w0(0, 2)
            nc.sync.dma_start(xsb[:, 0, 0:NT], xp[:, 0, 0:NT])
            nc.sync.dma_start(xsb[:, 0, NT:T], xp[:, 0, NT:T])
            xq = 1
            issue_w0(2, 6)
            issue_x(1)
            issue_x(1)
            issue_w0(6, 11)
            issue_x(1)
            issue_x(1)
            issue_w0(11, DT)
            issue_x(1)
            issue_x(1)
            issue_x(1)
            w_nxt = load_w(1)
            while xq < DT:
                issue_x(2)

            g = gpool.tile([P, HT, T], BF16, tag="g")

            def mm_quarters(ps, wsb, dt_i, th, start):
                # N=128 quarter MMs: denser early PE stream (same psum bank,
                # only the group's very first MM carries start=True)
                for q in range(4):
                    qs = slice(th * NT + q * P, th * NT + (q + 1) * P)
                    ps_q = slice(q * P, (q + 1) * P)
                    nc.tensor.matmul(
                        ps[:, ps_q],
                        lhsT=wsb[:, dt_i],
                        rhs=xsb[:, dt_i, qs],
                        start=(start and q == 0),
                        stop=False,
                        skip_group_check=True,
                    )

            for ht in range(HT):
                if ht == 0:
                    wsb = w_pre
                elif ht == 1:
                    wsb = w_nxt
                else:
                    wsb = w_cur
                if 1 <= ht < HT - 1:
                    w_cur = load_w(ht + 1)
                w1sb, w3sb = wsb[:, 0], wsb[:, 1]
                ps1 = [
                    pspool.tile([P, NT], F32, tag="ps", bufs=7, name=f"ps1_{th}")
                    for th in range(TH)
                ]
                ps3 = [
                    pspool.tile([P, NT], F32, tag="ps", bufs=7, name=f"ps3_{th}")
                    for th in range(TH)
                ]
                # ht=0: all 4 groups advance per dt chunk, so each x chunk is
                # consumed once as it arrives (~290 GB/s demand, matches DMA).
                # ht>0: all w1 groups then all w3 — frees ps1 banks (silu-only
                # readers) early so the next ht never waits on psum rotation.
                if ht == 0:
                    for dt_i in range(DT):
                        if dt_i < 4:
                            # th-major so the first 8 quarter-MM bursts only
                            # need the t<512 half of the x chunk
                            for th in range(TH):
                                for ps_pair, wsb in ((ps1, w1sb), (ps3, w3sb)):
                                    mm_quarters(
                                        ps_pair[th], wsb, dt_i, th,
                                        start=(dt_i == 0),
                                    )
                            continue
                        for ps_pair, wsb in ((ps1, w1sb), (ps3, w3sb)):
                            for th in range(TH):
                                ts = slice(th * NT, (th + 1) * NT)
                                nc.tensor.matmul(
                                    ps_pair[th],
                                    lhsT=wsb[:, dt_i],
                                    rhs=xsb[:, dt_i, ts],
                                    start=(dt_i == 0),
                                    stop=(dt_i == DT - 1),
                                    skip_group_check=True,
                                )
                else:
                    for ps_pair, wsb in ((ps1, w1sb), (ps3, w3sb)):
                        for dt_i in range(DT):
                            for th in range(TH):
                                ts = slice(th * NT, (th + 1) * NT)
                                nc.tensor.matmul(
                                    ps_pair[th],
                                    lhsT=wsb[:, dt_i],
                                    rhs=xsb[:, dt_i, ts],
                                    start=(dt_i == 0),
                                    stop=(dt_i == DT - 1),
                                )
                for th in range(TH):
                    ts = slice(th * NT, (th + 1) * NT)
                    sil = spool.tile([P, NT], F32, tag="sil")
                    nc.scalar.activation(
                        sil, ps1[th], mybir.ActivationFunctionType.Silu
                    )
                    nc.vector.tensor_mul(out=g[:, ht, ts], in0=sil, in1=ps3[th])

            # phase 2: out^T[dtt] = sum_ht w2tile.T @ g, full-K psum groups
            for dtt in range(DTT):
                w2sb = w2pool.tile([P, HT, P], BF16, tag="w2")
                nc.sync.dma_start(w2sb, w2p[dtt])
                po = [
                    pspool.tile([P, NT], F32, tag="ps", bufs=7, name=f"po_{th}")
                    for th in range(TH)
                ]
                for ht in range(HT):
                    for th in range(TH):
                        ts = slice(th * NT, (th + 1) * NT)
                        nc.tensor.matmul(
                            po[th],
                            lhsT=w2sb[:, ht],
                            rhs=g[:, ht, ts],
                            start=(ht == 0),
                            stop=(ht == HT - 1),
                        )
                stage = stpool.tile([P, T], F32, tag="stage")
                # drain the two t-halves on both engines in parallel
                nc.scalar.copy(stage[:, 0:NT], po[0])
                nc.vector.tensor_copy(out=stage[:, NT:T], in_=po[1])
                dsl = slice(dtt * P, (dtt + 1) * P)
                if dtt < DTT - 1:
                    nc.sync.dma_start(outT[dsl], stage)
                else:
                    # split the last DMA so the tail drains in halves
                    nc.sync.dma_start(outT[dsl, 0:NT], stage[:, 0:NT])
                    nc.sync.dma_start(outT[dsl, NT:T], stage[:, NT:T])
    nc.compile()
    return nc


def _pack_inputs(x, w1, w2, w3):
    """Per-expert host-side packing into DMA-linear bf16 layouts."""
    bf = ml_dtypes.bfloat16
    in_maps = []
    for e in range(E):
        xe = np.asarray(x[e], dtype=np.float32).astype(bf)
        w1e = np.asarray(w1[e], dtype=np.float32).astype(bf)
        w2e = np.asarray(w2[e], dtype=np.float32).astype(bf)
        w3e = np.asarray(w3[e], dtype=np.float32).astype(bf)
        # xp[p, dt, t] = x[t, dt*128+p]  (matches the xsb SBUF layout)
        xpk = np.ascontiguousarray(xe.reshape(T, DT, P).transpose(2, 1, 0))
        # wp[ht, p, 0, dt, h] = w1[dt*128+p, ht*128+h]
        # wp[ht, p, 1, dt, h] = w3[ht*128+h, dt*128+p]
        w1pk = w1e.reshape(DT, P, HT, P).transpose(2, 1, 0, 3)
        w3pk = w3e.reshape(HT, P, DT, P).transpose(0, 3, 2, 1)
        wpk = np.ascontiguousarray(np.stack([w1pk, w3pk], axis=2))
        # w2p[dtt, p, ht, d] = w2[dtt*128+d, ht*128+p]  (partition-first tile)
        w2pk = np.ascontiguousarray(
            w2e.reshape(DTT, P, HT, P).transpose(0, 3, 2, 1)
        )
        in_maps.append({"xp": xpk, "wp": wpk, "w2p": w2pk})
    return in_maps


def kernel(x, w1, w2, w3, _trace=False, _trace_kwargs=None):
    if "nc" not in _CACHE:
        _CACHE["nc"] = _build_nc()
    nc = _CACHE["nc"]
    in_maps = _pack_inputs(x, w1, w2, w3)
    kw = {}
    if _trace:
        kw = {"trace": True}
        if _trace_kwargs:
            kw.update(_trace_kwargs)
    res = run_bass_kernel_spmd(nc, in_maps, core_ids=list(range(E)), **kw)
    out = np.empty((E, T, D), dtype=np.float32)
    for e in range(E):
        out[e] = res.results[e]["outT"].T
    if _trace:
        _CACHE["last_results"] = res
    return out
